# revision 11
# baseline (speedup 1.0000x reference)
"""Conv4d (3,3,3,3) kernel for Trainium2, 8 NeuronCores.

Problem: x (2,24,16,16,48,48) * weight (48,24,3,3,3,3) + bias3d.sum(0)
      -> out (2,48,14,14,46,46), stride 1, no padding.

Strategy
--------
Sharding: 8 cores = (batch 2) x (ol-block 2) x (od-block 2). Each core owns a
7x7 block of (ol, od) output planes (49 tasks).

Per task: implicit GEMM. Contraction rows = (lo, do, ci) = 216 (+1 bias row),
packed on the host into xs[t, 217, 48, 48] where row r = (lo*3+do)*24+ci is
the input plane x[b, ci, ol+lo, od+do, :, :]; row 216 is all-ones. For each
of the 9 (ho, wo) kernel offsets the moving operand is the same SBUF-resident
tile sliced [k, oh0+ho : oh0+ho+rows, wo : wo+46]; all offsets accumulate
into one PSUM tile of output rows [48, rows, 46]. Bias is weight row 216
(offset (0,0) only) against the ones row.

M=48 output channels only fills col strips 0-1 of the 128-wide PE array, so
output-row chunks are processed in column-split PAIRS: partner chunk's
matmuls go to psum partitions 64-111 with tile_position (0, 64) — the two
streams run on disjoint 32x32 sub-array column groups concurrently, ~2x the
serial rate. Chunks (8,8,8,8,7,7) pair as (0,1) (2,3) (4,5) so no chunk is
left unpaired.

dtype fp16: 1 col/cycle on the PE, ~3e-4 scale-relative error after fp32
PSUM accumulation (weights/activations well inside fp16 range).
"""

import os
import sys

if "/opt/trn_rl_repo" not in sys.path:
    sys.path.insert(0, "/opt/trn_rl_repo")

from contextlib import nullcontext

import numpy as np

from concourse import bacc, bass, tile
from concourse.bass_utils import run_bass_kernel_spmd

mybir = bass.mybir

B, CI, CO = 2, 24, 48
L, D, H, W = 16, 16, 48, 48
OL, OD, OH, OW = 14, 14, 46, 46
N_TASKS = 49  # 7x7 (ol, od) planes per core
KROWS = 217  # (lo,do,ci) contraction rows + ones row
KPAD = 256  # xs padded to 2x128 rows: k2 zero-padded so every matmul is K=128
K2PAD = os.environ.get("CONV_K2PAD", "0") == "1"
KSPLIT = 128  # k1 = rows 0:128, k2 = rows 128:217
FPAD = 2306  # flat 48x48 plane + 2 pad elems for the largest shifted window

COLSPLIT = os.environ.get("CONV_COLSPLIT", "1") == "1"
if COLSPLIT:
    CHUNK_ROWS = (8, 8, 8, 8, 7, 7)
    CHUNK_OH0 = (0, 8, 16, 24, 32, 39)
    PAIRS = ((0, 1), (2, 3), (4, 5))
else:
    CHUNK_ROWS = (10, 10, 10, 10, 6)
    CHUNK_OH0 = (0, 10, 20, 30, 40)
    PAIRS = ((0, 1), (2, 3), (4,))

_DTYPE_NAMES = {
    "f32r": mybir.dt.float32r,
    "bf16": mybir.dt.bfloat16,
    "f16": mybir.dt.float16,
    "f32": mybir.dt.float32,
}
DTYPE = _DTYPE_NAMES[os.environ.get("CONV_DTYPE", "f16")]
X_BUFS = int(os.environ.get("CONV_XBUFS", "4"))
PS_BUFS = int(os.environ.get("CONV_PSBUFS", "8"))
O_BUFS = int(os.environ.get("CONV_OBUFS", "3"))
# f16 output staging: halves DVE-copy write + out-DMA bytes (~2.4e-4 rel err,
# well under budget); host upcasts
ODTYPE_NAME = os.environ.get("CONV_ODTYPE", "f16")
# bench-only bisection knobs (break correctness!)
NOLOAD = os.environ.get("CONV_NOLOAD", "0") == "1"  # one static k-tile pair
NOEPI = os.environ.get("CONV_NOEPI", "0") == "1"  # epilogue on last task only


def _np_dtype():
    return mybir.dt.np(DTYPE)


def build_program(n_tasks: int = N_TASKS, repeat: int = 1):
    nc = bacc.Bacc()
    f32 = mybir.dt.float32
    odt = _DTYPE_NAMES[ODTYPE_NAME]
    k2rows = (KPAD if K2PAD else KROWS) - KSPLIT

    xs_d = nc.dram_tensor(
        "xs", [n_tasks, KPAD if K2PAD else KROWS, FPAD], DTYPE, kind="ExternalInput"
    )
    out_d = nc.dram_tensor("out", [n_tasks, CO, OH, OW], odt, kind="ExternalOutput")
    w1_d = nc.dram_tensor("w1", [KSPLIT, 9, CO], DTYPE, kind="ExternalInput")
    w2_d = nc.dram_tensor("w2", [k2rows, 9, CO], DTYPE, kind="ExternalInput")

    with tile.TileContext(nc) as tc:
        with (
            tc.tile_pool(name="wpool", bufs=1) as wpool,
            tc.tile_pool(name="xpool", bufs=X_BUFS) as xpool,
            tc.tile_pool(name="opool", bufs=O_BUFS) as opool,
            tc.tile_pool(name="pspool", bufs=PS_BUFS, space="PSUM") as pspool,
            tc.For_i(0, repeat, 1) if repeat > 1 else nullcontext(),
        ):
            w1s = wpool.tile([KSPLIT, 9, CO], DTYPE)
            w2s = wpool.tile([k2rows, 9, CO], DTYPE)
            nc.sync.dma_start(out=w1s[:], in_=w1_d[:])
            nc.sync.dma_start(out=w2s[:], in_=w2_d[:])

            if NOLOAD:
                k1s = xpool.tile([KSPLIT, FPAD], DTYPE, tag="k1")
                k2s = xpool.tile([k2rows, FPAD], DTYPE, tag="k2")
                nc.gpsimd.dma_start(out=k1s[:], in_=xs_d[0, 0:KSPLIT])
                nc.gpsimd.dma_start(out=k2s[:], in_=xs_d[0, KSPLIT : KSPLIT + k2rows])
            for t in range(n_tasks):
                if NOLOAD:
                    k1, k2 = k1s, k2s
                else:
                    k1 = xpool.tile([KSPLIT, FPAD], DTYPE, tag="k1")
                    k2 = xpool.tile([k2rows, FPAD], DTYPE, tag="k2")
                    # gpsimd SWDGE: async descriptor gen on Q7 — sync-queue
                    # (SP HWDGE) loads measurably serialize against the PE
                    # semaphore chain and cost ~9 us/task of stream stalls
                    nc.gpsimd.dma_start(out=k1[:], in_=xs_d[t, 0:KSPLIT])
                    nc.gpsimd.dma_start(out=k2[:], in_=xs_d[t, KSPLIT : KSPLIT + k2rows])
                # one output staging tile per task; all chunk copies land here
                # (j=1 copies re-base partitions 64-111 -> 0-47) so the store
                # is a single large HWDGE DMA instead of 6 SWDGE ones
                o_task = opool.tile([CO, OH, OW], odt, tag="o")

                # weight-stationary order: one (idx, kt) weight pair serves all
                # 6 chunk matmuls before the next LDWEIGHTS, so the LDW (which
                # can't overlap in-flight full-K matmuls) costs 1/6 as many
                # pipeline bubbles and the A/B col-tile streams stay concurrent
                ps_l, p0_l = [], []
                for pair in PAIRS:
                    for j, c in enumerate(pair):
                        rows = CHUNK_ROWS[c]
                        # psum tile is one full bank (512 f32) per partition;
                        # matmul writes it as a flat [48, rows*46] AP.
                        # colsplit partner j=1 sits at partitions 64-111 on
                        # col strips 2-3 of the PE array.
                        p0 = 64 * j if COLSPLIT else 0
                        ps_full = pspool.tile([128, 512], f32, tag="ps")
                        ps_l.append(ps_full[p0 : p0 + CO, 0 : rows * W])
                        p0_l.append(p0)

                for idx in range(9):
                    ho, wo = divmod(idx, 3)
                    for kt, (ks, ws) in enumerate(((k1, w1s), (k2, w2s))):
                        for c in range(len(CHUNK_ROWS)):
                            rows = CHUNK_ROWS[c]
                            oh0 = CHUNK_OH0[c]
                            # contiguous pitch-48 window: 2 junk columns per
                            # row stream through and are cropped in the copy —
                            # strided 3D rhs reads measurably break dual-stream
                            # col-tiling concurrency
                            off = (oh0 + ho) * W + wo
                            rhs = ks[:, off : off + rows * W]
                            nc.tensor.matmul(
                                ps_l[c],
                                lhsT=ws[:, idx, :],
                                rhs=rhs,
                                start=(idx == 0 and kt == 0),
                                stop=(idx == 8 and kt == 1),
                                tile_position=(0, p0_l[c]),
                            )

                for c in range(len(CHUNK_ROWS)):
                    if NOEPI and t != n_tasks - 1:
                        continue
                    rows = CHUNK_ROWS[c]
                    oh0 = CHUNK_OH0[c]
                    nc.vector.tensor_copy(
                        out=o_task[:, oh0 : oh0 + rows, :],
                        in_=ps_l[c].rearrange("p (r w) -> p r w", w=W)[:, :, 0:OW],
                    )
                if not (NOEPI and t != n_tasks - 1):
                    nc.scalar.dma_start(out=out_d[t], in_=o_task[:])
    nc.finalize()
    return nc


def make_in_maps(x, weight, bias3d, n_tasks: int = N_TASKS):
    """Host-side shard + repack into the per-task packed-row layout."""
    npdt = _np_dtype()
    x = np.asarray(x, np.float32)
    weight = np.asarray(weight, np.float32)
    bias3d = np.asarray(bias3d, np.float32)

    # W[(lo*3+do)*24+ci, ho*3+wo, co] = weight[co, ci, lo, do, ho, wo]
    Wr = np.ascontiguousarray(np.transpose(weight, (2, 3, 1, 4, 5, 0))).reshape(
        216, 9, CO
    )
    nk = KPAD if K2PAD else KROWS
    Wfull = np.zeros((nk, 9, CO), np.float32)
    Wfull[:216] = Wr
    Wfull[216, 0, :] = bias3d.sum(axis=0)
    w1 = np.ascontiguousarray(Wfull[:KSPLIT]).astype(npdt)
    w2 = np.ascontiguousarray(Wfull[KSPLIT:]).astype(npdt)

    in_maps = []
    for c in range(8):
        b, lb, db = c // 4, (c // 2) % 2, c % 2
        slab = np.ascontiguousarray(
            x[b, :, 7 * lb : 7 * lb + 9, 7 * db : 7 * db + 9]
        )  # (24, 9, 9, 48, 48)
        s_ci, s_l, s_d, s_h, s_w = slab.strides
        # V[l0, d0, lo, do, ci, h, w] = slab[ci, l0+lo, d0+do, h, w]
        V = np.lib.stride_tricks.as_strided(
            slab,
            shape=(7, 7, 3, 3, CI, H, W),
            strides=(s_l, s_d, s_l, s_d, s_ci, s_h, s_w),
        )
        xs = np.zeros((N_TASKS, KPAD if K2PAD else KROWS, FPAD), np.float32)
        xs[:, :216, : H * W] = V.reshape(N_TASKS, 216, H * W)
        xs[:, 216] = 1.0
        in_maps.append({"xs": xs[:n_tasks].astype(npdt), "w1": w1, "w2": w2})
    return in_maps


def assemble_output(results):
    out = np.empty((B, CO, OL, OD, OH, OW), np.float32)
    for c in range(8):
        b, lb, db = c // 4, (c // 2) % 2, c % 2
        r = np.asarray(results[c]["out"]).reshape(7, 7, CO, OH, -1)[..., :OW]
        out[b, :, 7 * lb : 7 * lb + 7, 7 * db : 7 * db + 7] = r.transpose(2, 0, 1, 3, 4)
    return out


_NC_CACHE = {}


def _get_program():
    if "nc" not in _NC_CACHE:
        _NC_CACHE["nc"] = build_program()
    return _NC_CACHE["nc"]


def kernel(x, weight, bias3d):
    nc = _get_program()
    in_maps = make_in_maps(x, weight, bias3d)
    res = run_bass_kernel_spmd(nc, in_maps, list(range(8))).results
    return assemble_output(res)
